# revision 13
# baseline (speedup 1.0000x reference)
"""Trainium2 Bass kernel for nn_AttentionBlock (B=4, S=2048, D=1024, H=16).

Sharding: 8 cores, core c -> (batch b = c//2, seq-half sh = c%2). Each core
computes attention for all 16 heads over its 1024 query rows against the full
2048 keys of its batch, plus the (doubled) output projection. No collectives.

Device-side layouts (T = transposed [feature, seq]):
  QT = (Xq @ Wq + bq)^T       [1024, 1024]   (per-core query rows)
  KT = (Xk @ Wk + bk)^T       [1024, 2048]
  V  =  Xv @ Wv (no bias)     [2048, 1024+ones]  keys on partitions
  scT_h = KT_h^T' .. -> scores^T [2048 keys, 1024 rows] per head
  E^T = exp(scT/8)  -> written raw to awT output; rowsum via ones-column of V
  attn^T = (V' E^T) * (1/rowsum) + bv ; out^T = Wo^T(Wo^T attn^T + bo) + bo
Host applies: aw[b,h,q,k] = awT[h,k,q] * rinv[h,q] and out = outT.T.
"""

import os
import sys

for _p in ("/opt/trn_rl_repo",):
    if os.path.isdir(_p) and _p not in sys.path:
        sys.path.insert(0, _p)

import numpy as np

import concourse.bass as bass
import concourse.mybir as mybir
import concourse.tile as tile
from concourse import bacc
from concourse.bass_utils import run_bass_kernel_spmd
from concourse.masks import make_identity

B, S, D, H = 4, 2048, 1024, 16
DK = D // H          # 64
RB = S // 2          # 1024 rows per core
NT = D // 128        # 8 feature tiles
KT16 = S // 128      # 16 key tiles
F32 = mybir.dt.float32
F32R = mybir.dt.float32r
MULT = mybir.AluOpType.mult
EXP = mybir.ActivationFunctionType.Exp

_CACHE = {}


def _r(ap):
    return ap.bitcast(F32R)


def _load_bias_sb(nc, pool, dram):
    """bias [1024] -> SBUF [128, 8] with b_sb[p, ct] = bias[ct*128 + p]."""
    t = pool.tile([128, NT], F32, name=f"b_{dram.name}", tag=f"b_{dram.name}")
    nc.sync.dma_start(t, dram.ap().rearrange("(ct p) -> p ct", p=128))
    return t


def _transpose_chunk(nc, tc, x_dram, row0, xin_pool, xt_pool, tr_psum, ident):
    """Load x[row0:row0+512, :1024] and return 8 SBUF tiles xt[dt] = [128, 512]
    with xt[dt][p, s] = x[row0+s, dt*128+p] (PE transposes)."""
    xin = []
    for si in range(4):
        t = xin_pool.tile([128, D], F32)
        nc.sync.dma_start(t, x_dram.ap()[row0 + si * 128 : row0 + (si + 1) * 128, :])
        xin.append(t)
    xts = []
    for dt in range(NT):
        xt = xt_pool.tile([128, 512], F32R)
        for si in range(4):
            tp = tr_psum.tile([128, 128], F32)
            nc.tensor.transpose(tp, xin[si][:, dt * 128 : (dt + 1) * 128], ident)
            nc.vector.tensor_copy(xt[:, si * 128 : (si + 1) * 128], tp)
        xts.append(xt)
    return xts


def _build():

    if "nc" in _CACHE:
        return _CACHE["nc"]
    nc = bacc.Bacc("TRN2", target_bir_lowering=False, debug=False)

    # Per-core external inputs
    q_in = nc.dram_tensor("q_rows", [RB, D], F32, kind="ExternalInput")
    k_in = nc.dram_tensor("k_full", [S, D], F32, kind="ExternalInput")
    v_in = nc.dram_tensor("v_full", [S, D], F32, kind="ExternalInput")
    wq_in = nc.dram_tensor("wq", [D, D], F32, kind="ExternalInput")
    wk_in = nc.dram_tensor("wk", [D, D], F32, kind="ExternalInput")
    wv_in = nc.dram_tensor("wv", [D, D], F32, kind="ExternalInput")
    wo_in = nc.dram_tensor("wo", [D, D], F32, kind="ExternalInput")
    bq_in = nc.dram_tensor("bq", [D], F32, kind="ExternalInput")
    bk_in = nc.dram_tensor("bk", [D], F32, kind="ExternalInput")
    bv_in = nc.dram_tensor("bv", [D], F32, kind="ExternalInput")
    bo_in = nc.dram_tensor("bo", [D], F32, kind="ExternalInput")

    # Outputs
    awT = nc.dram_tensor("awT", [H, S, RB], F32, kind="ExternalOutput")
    rinv_o = nc.dram_tensor("rinv", [H, RB], F32, kind="ExternalOutput")
    outT = nc.dram_tensor("outT", [D, RB], F32, kind="ExternalOutput")

    # DRAM scratch
    qt_s = nc.dram_tensor("qt_s", [NT, 128, RB], F32R, kind="Internal")
    kt_s = nc.dram_tensor("kt_s", [NT, 128, S], F32R, kind="Internal")
    va_s = nc.dram_tensor("va_s", [KT16, 128, H, DK + 1], F32R, kind="Internal")

    with tile.TileContext(nc) as tc:
        with (
            tc.tile_pool(name="consts", bufs=1) as consts,
            tc.tile_pool(name="att", bufs=1) as att_pool,
            tc.tile_pool(name="et", bufs=10) as et_pool,
        ):
            ident = consts.tile([128, 128], F32)
            make_identity(nc, ident)
            ones64 = consts.tile([1, DK], F32)
            nc.vector.memset(ones64, 1.0)
            bq_sb = _load_bias_sb(nc, consts, bq_in)
            bk_sb = _load_bias_sb(nc, consts, bk_in)
            bv_sb = _load_bias_sb(nc, consts, bv_in)
            bo_sb = _load_bias_sb(nc, consts, bo_in)
            att = [att_pool.tile([128, RB], F32R, name=f"att{i}", tag=f"att{i}") for i in range(NT)]

            # ---------------- Phase A: projections ----------------
            for which, x_dram, w_dram, b_sb, nchunk in (
                ("k", k_in, wk_in, bk_sb, 4),
                ("q", q_in, wq_in, bq_sb, 2),
                ("v", v_in, wv_in, None, 4),
            ):
                dst = {"q": qt_s, "k": kt_s}.get(which)
                with (
                    tc.tile_pool(name=f"w_{which}", bufs=1) as w_pool,
                    tc.tile_pool(name=f"xin_{which}", bufs=4) as xin_pool,
                    tc.tile_pool(name=f"xt_{which}", bufs=8) as xt_pool,
                    tc.tile_pool(name=f"st_{which}", bufs=4) as st_pool,
                    tc.tile_pool(name=f"trp_{which}", bufs=2, space="PSUM") as trp,
                    tc.tile_pool(name=f"pp_{which}", bufs=2, space="PSUM") as pp,
                ):
                    w_sb = []
                    for kt in range(NT):
                        t = w_pool.tile([128, D], F32R, tag=f"w{kt}")
                        nc.sync.dma_start(t, w_dram.ap()[kt * 128 : (kt + 1) * 128, :].bitcast(F32R))
                        w_sb.append(t)
                    for ci in range(nchunk):
                        xts = _transpose_chunk(
                            nc, tc, x_dram, ci * 512, xin_pool, xt_pool, trp, ident
                        )
                        if which in ("q", "k"):
                            for ct in range(NT):
                                ps = pp.tile([128, 512], F32)
                                for kt in range(NT):
                                    nc.tensor.matmul(
                                        ps,
                                        w_sb[kt][:, ct * 128 : (ct + 1) * 128],
                                        xts[kt],
                                        start=(kt == 0),
                                        stop=(kt == NT - 1),
                                    )
                                st = st_pool.tile([128, 512], F32R)
                                nc.vector.tensor_scalar_add(
                                    st, ps, b_sb[:, ct : ct + 1]
                                )
                                nc.sync.dma_start(
                                    dst.ap()[ct, :, ci * 512 : (ci + 1) * 512], st
                                )
                        else:  # v: natural [keys, dims] + ones col per head
                            for kt4 in range(4):
                                ktile = ci * 4 + kt4
                                for dc in range(2):
                                    ps = pp.tile([128, 512], F32)
                                    for kt in range(NT):
                                        nc.tensor.matmul(
                                            ps,
                                            xts[kt][:, kt4 * 128 : (kt4 + 1) * 128],
                                            w_sb[kt][:, dc * 512 : (dc + 1) * 512],
                                            start=(kt == 0),
                                            stop=(kt == NT - 1),
                                        )
                                    st = st_pool.tile([128, 8 * (DK + 1)], F32R, tag="vst")
                                    stv = st.rearrange("p (h f) -> p h f", h=8)
                                    nc.vector.tensor_copy(
                                        stv[:, :, 0:DK],
                                        ps.rearrange("p (h f) -> p h f", f=DK),
                                    )
                                    nc.vector.memset(stv[:, :, DK : DK + 1].bitcast(F32), 1.0)
                                    nc.sync.dma_start(
                                        va_s.ap()[ktile, :, dc * 8 : (dc + 1) * 8, :], st
                                    )

            # ---------------- Phase B: attention per head ----------------
            with (
                tc.tile_pool(name="kth", bufs=2) as kth_pool,
                tc.tile_pool(name="qth", bufs=2) as qth_pool,
                tc.tile_pool(name="vh", bufs=2) as vh_pool,
                tc.tile_pool(name="rinv", bufs=2) as rinv_pool,
                tc.tile_pool(name="bcs", bufs=2) as bcs_pool,
                tc.tile_pool(name="scp", bufs=2, space="PSUM") as sc_psum,
                tc.tile_pool(name="pvp", bufs=1, space="PSUM") as pv_psum,
                tc.tile_pool(name="bcp", bufs=2, space="PSUM") as bc_psum,
            ):
                for h in range(H):
                    ct_h, ho = h // 2, (h % 2) * DK
                    kth = kth_pool.tile([DK, S], F32R)
                    for ci in range(4):
                        nc.sync.dma_start(
                            kth[:, ci * 512 : (ci + 1) * 512],
                            kt_s.ap()[ct_h, ho : ho + DK, ci * 512 : (ci + 1) * 512],
                        )
                    qth = qth_pool.tile([DK, RB], F32R)
                    for rc in range(2):
                        nc.sync.dma_start(
                            qth[:, rc * 512 : (rc + 1) * 512],
                            qt_s.ap()[ct_h, ho : ho + DK, rc * 512 : (rc + 1) * 512],
                        )
                    vh = vh_pool.tile([128, KT16, DK + 1], F32R)
                    for g in range(4):
                        nc.sync.dma_start(
                            vh[:, g * 4 : (g + 1) * 4, :],
                            va_s.ap()[g * 4 : (g + 1) * 4, :, h, :].rearrange(
                                "kt kp f -> kp kt f"
                            ),
                        )
                    rinvT = rinv_pool.tile([1, RB], F32)
                    pvps = [pv_psum.tile([DK + 1, 512], F32, name=f"pv{rc}", tag=f"pv{rc}") for rc in range(2)]
                    for half in range(2):
                        ets = []
                        for kt in range(half * 8, half * 8 + 8):
                            scp = sc_psum.tile([128, RB], F32)
                            for rc in range(2):
                                nc.tensor.matmul(
                                    scp[:, rc * 512 : (rc + 1) * 512],
                                    kth[:, kt * 128 : (kt + 1) * 128],
                                    qth[:, rc * 512 : (rc + 1) * 512],
                                    start=True,
                                    stop=True,
                                )
                            et = et_pool.tile([128, RB], F32R)
                            nc.scalar.activation(et, scp, EXP, scale=1.0 / np.sqrt(DK))
                            nc.sync.dma_start(
                                awT.ap()[h, kt * 128 : (kt + 1) * 128, :].bitcast(F32R), et
                            )
                            ets.append(et)
                        for i, kt in enumerate(range(half * 8, half * 8 + 8)):
                            for rc in range(2):
                                nc.tensor.matmul(
                                    pvps[rc],
                                    vh[:, kt, :],
                                    ets[i][:, rc * 512 : (rc + 1) * 512],
                                    start=(kt == 0),
                                    stop=(kt == KT16 - 1),
                                )
                    for rc in range(2):
                        sl = slice(rc * 512, (rc + 1) * 512)
                        nc.vector.reciprocal(rinvT[:, sl], pvps[rc][DK : DK + 1, :])
                        bcp = bc_psum.tile([DK, 512], F32)
                        nc.tensor.matmul(
                            bcp, ones64, rinvT[:, sl], start=True, stop=True
                        )
                        bc_sb = bcs_pool.tile([DK, 512], F32)
                        nc.vector.tensor_copy(bc_sb, bcp)
                        nc.vector.tensor_tensor(
                            att[ct_h][ho : ho + DK, sl], pvps[rc][0:DK, :], bc_sb, MULT
                        )
                        nc.vector.tensor_scalar_add(
                            att[ct_h][ho : ho + DK, sl],
                            att[ct_h][ho : ho + DK, sl],
                            bv_sb[ho : ho + DK, ct_h : ct_h + 1],
                        )
                    nc.sync.dma_start(rinv_o.ap()[h : h + 1, :], rinvT[0:1, :])

            # ---------------- Phase C: double output projection ----------------
            with (
                tc.tile_pool(name="wo", bufs=1) as wo_pool,
                tc.tile_pool(name="a1", bufs=1) as a1_pool,
                tc.tile_pool(name="ost", bufs=4) as ost_pool,
                tc.tile_pool(name="cp", bufs=4, space="PSUM") as cp,
            ):
                wo_sb = []
                for kt in range(NT):
                    t = wo_pool.tile([128, D], F32R, tag=f"wo{kt}")
                    nc.sync.dma_start(t, wo_in.ap()[kt * 128 : (kt + 1) * 128, :].bitcast(F32R))
                    wo_sb.append(t)
                a1 = [a1_pool.tile([128, RB], F32R, name=f"a1{i}", tag=f"a1{i}") for i in range(NT)]
                for src, is_final in ((att, False), (a1, True)):
                    for ct in range(NT):
                        for rc in range(2):
                            ps = cp.tile([128, 512], F32)
                            for kt in range(NT):
                                nc.tensor.matmul(
                                    ps,
                                    wo_sb[kt][:, ct * 128 : (ct + 1) * 128],
                                    src[kt][:, rc * 512 : (rc + 1) * 512],
                                    start=(kt == 0),
                                    stop=(kt == NT - 1),
                                )
                            if not is_final:
                                nc.vector.tensor_scalar_add(
                                    a1[ct][:, rc * 512 : (rc + 1) * 512],
                                    ps,
                                    bo_sb[:, ct : ct + 1],
                                )
                            else:
                                st = ost_pool.tile([128, 512], F32)
                                nc.vector.tensor_scalar_add(
                                    st, ps, bo_sb[:, ct : ct + 1]
                                )
                                nc.sync.dma_start(
                                    outT.ap()[
                                        ct * 128 : (ct + 1) * 128,
                                        rc * 512 : (rc + 1) * 512,
                                    ],
                                    st,
                                )

    nc.compile()
    _CACHE["nc"] = nc
    return nc


def kernel(values, keys, queries, Wq, bq, Wk, bk, Wv, bv, Wo, bo):
    nc = _build()
    f = lambda a: np.ascontiguousarray(np.asarray(a), dtype=np.float32)
    values, keys, queries = f(values), f(keys), f(queries)
    Wq, Wk, Wv, Wo = f(Wq), f(Wk), f(Wv), f(Wo)
    bq, bk, bv, bo = f(bq), f(bk), f(bv), f(bo)
    in_maps = []
    for c in range(8):
        b, sh = c // 2, c % 2
        in_maps.append(
            {
                "q_rows": f(queries[b, sh * RB : (sh + 1) * RB, :]),
                "k_full": keys[b],
                "v_full": values[b],
                "wq": Wq, "wk": Wk, "wv": Wv, "wo": Wo,
                "bq": bq, "bk": bk, "bv": bv, "bo": bo,
            }
        )
    res = run_bass_kernel_spmd(nc, in_maps, core_ids=list(range(8)))
    _CACHE["last_results"] = res
    out = np.empty((B, S, D), np.float32)
    aw = np.empty((B, H, S, S), np.float32)
    for c, r in enumerate(res.results):
        b, sh = c // 2, c % 2
        out[b, sh * RB : (sh + 1) * RB, :] = r["outT"].T
        aw[b, :, sh * RB : (sh + 1) * RB, :] = (
            r["awT"].transpose(0, 2, 1) * r["rinv"][:, :, None]
        )
    return (out, aw)
